# revision 27
# baseline (speedup 1.0000x reference)
"""Chamfer-loss min/argmin kernel for Trainium2 (8 NeuronCores).

Problem: preds [4, 8192, 3], gts [4, 8192, 3] fp32.
P[b, n, m] = ||gts[b,n]||^2 + ||preds[b,m]||^2 - 2 <gts[b,n], preds[b,m]>
Outputs: (min over n [4,8192], min over m [4,8192],
          argmin over n int32, argmin over m int32).

Sharding: 8 cores = 4 batches x 2 halves of the gts (n) axis. Each core
holds full preds for its batch and a 4096-row slice of gts. Per-gt-row
results (min over m) are final; per-pred-row results (min over n) are
partial over the n-slice and combined on the host.

Device kernel per core (both directions, roles swapped):
 - K=4 fp32 matmuls (rows [-2x0,-2x1,-2x2,1] x [y0,y1,y2,ry]) produce
   Q = -2<x,y> + ry_free directly in PSUM.  Matmuls are quad-packed with
   tile_position row groups (4 concurrent small-K matmuls) filling one
   [128, 2048] PSUM group per quad.
 - DVE tensor_scalar stages each PSUM group to SBUF while adding the
   per-partition norm (P = Q + rx) and min-reducing into a group accum.
 - max_index finds the first-occurrence argmin (jnp tie semantics).
"""

import functools

import numpy as np

BS, N, M, D = 4, 8192, 8192, 3
NSL = N // 2  # gts rows per core
K = 4  # contraction: 3 coords + free-side norm
N_CORES = 8
GROUP = 2048  # PSUM group: 4 banks of 512 fp32


def _build_nc(nsl, m, reps=1):
    import contextlib

    import concourse.bacc as bacc
    import concourse.mybir as mybir
    import concourse.tile as tile

    f32 = mybir.dt.float32
    i32 = mybir.dt.int32
    u32 = mybir.dt.uint32

    nc = bacc.Bacc("TRN2", target_bir_lowering=False, debug=False)

    ga = nc.declare_dram_parameter("ga", [2 * K, nsl], f32, isOutput=False)
    pa = nc.declare_dram_parameter("pa", [2 * K, m], f32, isOutput=False)
    rxc = nc.declare_dram_parameter("rxc", [128, nsl // 128], f32, isOutput=False)
    ryc = nc.declare_dram_parameter("ryc", [128, m // 128], f32, isOutput=False)
    gmin_o = nc.declare_dram_parameter("gmin", [128, nsl // 128], f32, isOutput=True)
    gidx_o = nc.declare_dram_parameter("gidx", [128, nsl // 128], i32, isOutput=True)
    pmin_o = nc.declare_dram_parameter("pmin", [128, m // 128], f32, isOutput=True)
    pidx_o = nc.declare_dram_parameter("pidx", [128, m // 128], i32, isOutput=True)

    with tile.TileContext(nc) as tc:
        with (
            tc.tile_pool(name="const", bufs=1) as const,
            tc.tile_pool(name="rows", bufs=2) as rows,
            tc.tile_pool(name="small", bufs=4) as small,
            tc.tile_pool(name="outs", bufs=1) as outs,
            tc.tile_pool(name="psum", bufs=2, space="PSUM") as psum,
        ):
            # operands replicated into the 4 PE row groups (partitions 32j,
            # 32-aligned as the matmul requires), one tensor per operand role
            ga_repL = const.tile([128, nsl], f32)
            ga_repR = const.tile([128, nsl], f32)
            pa_repR = const.tile([128, m], f32)
            pa_repL = const.tile([128, m], f32)
            for j in range(4):
                nc.sync.dma_start(ga_repL[32 * j : 32 * j + K, :], ga[0:K, :])
                nc.sync.dma_start(ga_repR[32 * j : 32 * j + K, :], ga[K : 2 * K, :])
                nc.sync.dma_start(pa_repR[32 * j : 32 * j + K, :], pa[0:K, :])
                nc.sync.dma_start(pa_repL[32 * j : 32 * j + K, :], pa[K : 2 * K, :])
            rx_sb = const.tile([128, nsl // 128], f32)
            ry_sb = const.tile([128, m // 128], f32)
            nc.sync.dma_start(rx_sb[:], rxc[:])
            nc.sync.dma_start(ry_sb[:], ryc[:])

            rep_loop = tc.For_i(0, reps, 1) if reps > 1 else contextlib.nullcontext()
            rep_loop.__enter__()

            gmin_sb = outs.tile([128, nsl // 128], f32)
            gidx_sb = outs.tile([128, nsl // 128], i32)
            pmin_sb = outs.tile([128, m // 128], f32)
            pidx_sb = outs.tile([128, m // 128], i32)

            def direction(n_chunks, lhs_rep, rhs_rep, rhs_len, norm_sb,
                          min_sb, idx_sb, tagp, pipelined=False):
                n_groups = rhs_len // GROUP

                def tail(row, tmins, ci):
                    gm = small.tile([128, 1], f32, tag=f"gm{tagp}")
                    nc.vector.tensor_reduce(
                        gm[:], tmins[:], mybir.AxisListType.X, mybir.AluOpType.min
                    )
                    gm8 = small.tile([128, 8], f32, tag=f"gm8{tagp}")
                    nc.any.tensor_copy(out=gm8[:], in_=gm.to_broadcast([128, 8]))
                    i8 = small.tile([128, 8], u32, tag=f"i8{tagp}")
                    nc.vector.max_index(out=i8[:], in_max=gm8[:], in_values=row[:])
                    nc.any.tensor_copy(out=min_sb[:, ci : ci + 1], in_=gm[:])
                    nc.any.tensor_copy(out=idx_sb[:, ci : ci + 1], in_=i8[:, 0:1])

                pending = None
                for ci in range(n_chunks):
                    row = rows.tile([128, rhs_len], f32, tag=f"row{tagp}")
                    tmins = small.tile([128, n_groups], f32, tag=f"tmins{tagp}")
                    for g in range(n_groups):
                        pt = psum.tile([128, GROUP], f32, tag="pt")
                        for j in range(4):
                            lhsT = lhs_rep[32 * j : 32 * j + K,
                                           ci * 128 : (ci + 1) * 128]
                            col0 = g * GROUP + j * 512
                            nc.tensor.matmul(
                                pt[:, j * 512 : (j + 1) * 512],
                                lhsT=lhsT,
                                rhs=rhs_rep[32 * j : 32 * j + K, col0 : col0 + 512],
                                start=True,
                                stop=True,
                                tile_position=(32 * j, 0),
                            )
                        nc.vector.tensor_scalar(
                            row[:, g * GROUP : (g + 1) * GROUP],
                            pt[:],
                            norm_sb[:, ci : ci + 1],
                            None,
                            op0=mybir.AluOpType.add,
                            op1=mybir.AluOpType.min,
                            accum_out=tmins[:, g : g + 1],
                        )
                    # software pipeline: the argmin tail of chunk ci-1 is
                    # emitted after chunk ci's stage ops so the PE never
                    # stalls on PSUM buffers behind a long max_index
                    if not pipelined:
                        tail(row, tmins, ci)
                    else:
                        if pending is not None:
                            tail(*pending)
                        pending = (row, tmins, ci)
                if pending is not None:
                    tail(*pending)

            # per-gt rows: min/argmin over preds (final)
            direction(nsl // 128, ga_repL, pa_repR, m, rx_sb, gmin_sb, gidx_sb,
                      "g", pipelined=True)
            # per-pred rows: min/argmin over the gts slice (partial)
            direction(m // 128, pa_repL, ga_repR, nsl, ry_sb, pmin_sb, pidx_sb, "p")

            nc.sync.dma_start(gmin_o[:], gmin_sb[:])
            nc.sync.dma_start(gidx_o[:], gidx_sb[:])
            nc.sync.dma_start(pmin_o[:], pmin_sb[:])
            nc.sync.dma_start(pidx_o[:], pidx_sb[:])

            rep_loop.__exit__(None, None, None)
    nc.finalize()
    return nc


@functools.lru_cache(maxsize=None)
def _get_nc(nsl, m, reps=1):
    return _build_nc(nsl, m, reps)


def _augment(preds_b, gts_bh):
    """Operands for the K=4 scheme.

    ga rows: [-2x0, -2x1, -2x2, 1]  (gts slice, [4, nsl])
    pa rows: [y0, y1, y2, ry]       (full preds, [4, m])
    matmul: Q[n, m] = -2<x_n, y_m> + ry[m];  P = Q + rx via tensor_scalar.
    rxc/ryc: norms laid out [128, len//128] column-per-chunk.
    """
    x = np.ascontiguousarray(gts_bh, dtype=np.float32)
    y = np.ascontiguousarray(preds_b, dtype=np.float32)
    nsl = x.shape[0]
    m = y.shape[0]
    rx = (x[:, 0] * x[:, 0] + x[:, 1] * x[:, 1] + x[:, 2] * x[:, 2]).astype(np.float32)
    ry = (y[:, 0] * y[:, 0] + y[:, 1] * y[:, 1] + y[:, 2] * y[:, 2]).astype(np.float32)
    ga = np.empty((2 * K, nsl), np.float32)
    ga[0:3] = (np.float32(-2.0) * x).T   # lhsT rows, per-gt
    ga[3] = 1.0
    ga[4:7] = ga[0:3]                    # rhs rows, per-pred
    ga[7] = rx
    pa = np.empty((2 * K, m), np.float32)
    pa[0:3] = y.T                        # rhs rows, per-gt
    pa[3] = ry
    pa[4:7] = y.T                        # lhsT rows, per-pred
    pa[7] = 1.0
    rxc = np.ascontiguousarray(rx.reshape(nsl // 128, 128).T)
    ryc = np.ascontiguousarray(ry.reshape(m // 128, 128).T)
    return ga, pa, rxc, ryc


@functools.lru_cache(maxsize=None)
def _get_dispatcher(nsl, m, reps=1):
    """Build the SPMD PJRT dispatcher once and cache it (the stock
    run_bass_via_pjrt re-traces jax.jit on every call)."""
    import jax
    import numpy as _np
    from jax.sharding import Mesh, PartitionSpec
    from jax.experimental.shard_map import shard_map
    import concourse.mybir as mybir
    from concourse import bass2jax

    bass2jax.install_neuronx_cc_hook()
    nc = _get_nc(nsl, m, reps)

    partition_name = nc.partition_id_tensor.name if nc.partition_id_tensor else None
    in_names, out_names, out_avals, zero_outs = [], [], [], []
    for alloc in nc.m.functions[0].allocations:
        if not isinstance(alloc, mybir.MemoryLocationSet):
            continue
        name = alloc.memorylocations[0].name
        if alloc.kind == "ExternalInput":
            if name != partition_name:
                in_names.append(name)
        elif alloc.kind == "ExternalOutput":
            shape = tuple(alloc.tensor_shape)
            dtype = mybir.dt.np(alloc.dtype)
            out_names.append(name)
            out_avals.append(jax.core.ShapedArray(shape, dtype))
            zero_outs.append(_np.zeros(shape, dtype))
    n_params = len(in_names)
    n_outs = len(out_avals)
    all_in_names = list(in_names) + list(out_names)
    if partition_name is not None:
        all_in_names.append(partition_name)
    donate = tuple(range(n_params, n_params + n_outs))

    def _body(*args):
        operands = list(args)
        if partition_name is not None:
            operands.append(bass2jax.partition_id_tensor())
        outs = bass2jax._bass_exec_p.bind(
            *operands,
            out_avals=tuple(out_avals),
            in_names=tuple(all_in_names),
            out_names=tuple(out_names),
            lowering_input_output_aliases=(),
            sim_require_finite=True,
            sim_require_nnan=True,
            nc=nc,
        )
        return tuple(outs)

    devices = jax.devices()[:N_CORES]
    mesh = Mesh(np.asarray(devices), ("core",))
    in_specs = (PartitionSpec("core"),) * (n_params + n_outs)
    out_specs = (PartitionSpec("core"),) * n_outs
    sharded = jax.jit(
        shard_map(_body, mesh=mesh, in_specs=in_specs, out_specs=out_specs,
                  check_rep=False),
        donate_argnums=donate,
        keep_unused=True,
    )

    def dispatch(in_maps):
        concat_in = [
            np.concatenate([np.asarray(in_maps[c][nm]) for c in range(N_CORES)], axis=0)
            for nm in in_names
        ]
        concat_zeros = [
            np.zeros((N_CORES * z.shape[0], *z.shape[1:]), z.dtype) for z in zero_outs
        ]
        out_arrs = sharded(*concat_in, *concat_zeros)
        return [
            {nm: np.asarray(out_arrs[i]).reshape(N_CORES, *out_avals[i].shape)[c]
             for i, nm in enumerate(out_names)}
            for c in range(N_CORES)
        ]

    return dispatch


def _make_in_maps(preds, gts):
    in_maps = []
    for c in range(N_CORES):
        b, h = c // 2, c % 2
        ga, pa, rxc, ryc = _augment(preds[b], gts[b, h * NSL : (h + 1) * NSL])
        in_maps.append({"ga": ga, "pa": pa, "rxc": rxc, "ryc": ryc})
    return in_maps


def kernel(preds, gts, mask):
    preds = np.asarray(preds, dtype=np.float32)
    gts = np.asarray(gts, dtype=np.float32)

    results = _get_dispatcher(NSL, M)(_make_in_maps(preds, gts))

    out_pmin = np.empty((BS, M), np.float32)
    out_gmin = np.empty((BS, N), np.float32)
    out_pidx = np.empty((BS, M), np.int32)
    out_gidx = np.empty((BS, N), np.int32)

    for b in range(BS):
        r0, r1 = results[2 * b], results[2 * b + 1]
        # per-gt rows (min over preds): each half is final
        for h, r in ((0, r0), (1, r1)):
            out_gmin[b, h * NSL : (h + 1) * NSL] = r["gmin"].T.reshape(NSL)
            out_gidx[b, h * NSL : (h + 1) * NSL] = r["gidx"].T.reshape(NSL)
        # per-pred rows: combine the two n-halves
        pm0 = r0["pmin"].T.reshape(M)
        pm1 = r1["pmin"].T.reshape(M)
        pi0 = r0["pidx"].T.reshape(M)
        pi1 = r1["pidx"].T.reshape(M)
        take1 = pm1 < pm0  # tie -> half 0 (lower gt index), first occurrence
        out_pmin[b] = np.where(take1, pm1, pm0)
        out_pidx[b] = np.where(take1, pi1 + NSL, pi0)

    return out_pmin, out_gmin, out_pidx, out_gidx


# revision 28
# speedup vs baseline: 1.0829x; 1.0829x over previous
"""Chamfer-loss min/argmin kernel for Trainium2 (8 NeuronCores).

Problem: preds [4, 8192, 3], gts [4, 8192, 3] fp32.
P[b, n, m] = ||gts[b,n]||^2 + ||preds[b,m]||^2 - 2 <gts[b,n], preds[b,m]>
Outputs: (min over n [4,8192], min over m [4,8192],
          argmin over n int32, argmin over m int32).

Sharding: 8 cores = 4 batches x 2 halves of the gts (n) axis. Each core
holds full preds for its batch and a 4096-row slice of gts. Per-gt-row
results (min over m) are final; per-pred-row results (min over n) are
partial over the n-slice and combined on the host.

Device kernel per core (both directions, roles swapped):
 - K=4 fp32 matmuls (rows [-2x0,-2x1,-2x2,1] x [y0,y1,y2,ry]) produce
   Q = -2<x,y> + ry_free directly in PSUM.  Matmuls are quad-packed with
   tile_position row groups (4 concurrent small-K matmuls) filling one
   [128, 2048] PSUM group per quad.
 - DVE tensor_scalar stages each PSUM group to SBUF while adding the
   per-partition norm (P = Q + rx) and min-reducing into a group accum.
 - max_index finds the first-occurrence argmin (jnp tie semantics).
"""

import functools

import numpy as np

BS, N, M, D = 4, 8192, 8192, 3
NSL = N // 2  # gts rows per core
K = 4  # contraction: 3 coords + free-side norm
N_CORES = 8
GROUP = 2048  # PSUM group: 4 banks of 512 fp32


def _build_nc(nsl, m, reps=1):
    import contextlib

    import concourse.bacc as bacc
    import concourse.mybir as mybir
    import concourse.tile as tile

    f32 = mybir.dt.float32
    i32 = mybir.dt.int32
    u32 = mybir.dt.uint32

    nc = bacc.Bacc("TRN2", target_bir_lowering=False, debug=False)

    ga = nc.declare_dram_parameter("ga", [2 * K, nsl], f32, isOutput=False)
    pa = nc.declare_dram_parameter("pa", [2 * K, m], f32, isOutput=False)
    rxc = nc.declare_dram_parameter("rxc", [128, nsl // 128], f32, isOutput=False)
    ryc = nc.declare_dram_parameter("ryc", [128, m // 128], f32, isOutput=False)
    gmin_o = nc.declare_dram_parameter("gmin", [128, nsl // 128], f32, isOutput=True)
    gidx_o = nc.declare_dram_parameter("gidx", [128, nsl // 128], i32, isOutput=True)
    pmin_o = nc.declare_dram_parameter("pmin", [128, m // 128], f32, isOutput=True)
    pidx_o = nc.declare_dram_parameter("pidx", [128, m // 128], i32, isOutput=True)

    with tile.TileContext(nc) as tc:
        with (
            tc.tile_pool(name="const", bufs=1) as const,
            tc.tile_pool(name="rows", bufs=1) as rows,
            tc.tile_pool(name="small", bufs=4) as small,
            tc.tile_pool(name="outs", bufs=1) as outs,
            tc.tile_pool(name="psum", bufs=2, space="PSUM") as psum,
        ):
            # operands replicated into the 4 PE row groups (partitions 32j,
            # 32-aligned as the matmul requires), one tensor per operand role
            ga_repL = const.tile([128, nsl], f32)
            ga_repR = const.tile([128, nsl], f32)
            pa_repR = const.tile([128, m], f32)
            pa_repL = const.tile([128, m], f32)
            for j in range(4):
                nc.sync.dma_start(ga_repL[32 * j : 32 * j + K, :], ga[0:K, :])
                nc.sync.dma_start(ga_repR[32 * j : 32 * j + K, :], ga[K : 2 * K, :])
                nc.sync.dma_start(pa_repR[32 * j : 32 * j + K, :], pa[0:K, :])
                nc.sync.dma_start(pa_repL[32 * j : 32 * j + K, :], pa[K : 2 * K, :])
            rx_sb = const.tile([128, nsl // 128], f32)
            ry_sb = const.tile([128, m // 128], f32)
            nc.sync.dma_start(rx_sb[:], rxc[:])
            nc.sync.dma_start(ry_sb[:], ryc[:])

            rep_loop = tc.For_i(0, reps, 1) if reps > 1 else contextlib.nullcontext()
            rep_loop.__enter__()

            gmin_sb = outs.tile([128, nsl // 128], f32)
            gidx_sb = outs.tile([128, nsl // 128], i32)
            pmin_sb = outs.tile([128, m // 128], f32)
            pidx_sb = outs.tile([128, m // 128], i32)

            def direction(n_chunks, lhs_rep, rhs_rep, rhs_len, norm_sb,
                          min_sb, idx_sb, tagp):
                n_groups = rhs_len // GROUP
                for ci in range(n_chunks):
                    row = rows.tile([128, rhs_len], f32, tag=f"row{tagp}")
                    tmins = small.tile([128, n_groups], f32, tag=f"tmins{tagp}")
                    for g in range(n_groups):
                        pt = psum.tile([128, GROUP], f32, tag="pt")
                        for j in range(4):
                            lhsT = lhs_rep[32 * j : 32 * j + K,
                                           ci * 128 : (ci + 1) * 128]
                            col0 = g * GROUP + j * 512
                            nc.tensor.matmul(
                                pt[:, j * 512 : (j + 1) * 512],
                                lhsT=lhsT,
                                rhs=rhs_rep[32 * j : 32 * j + K, col0 : col0 + 512],
                                start=True,
                                stop=True,
                                tile_position=(32 * j, 0),
                            )
                        nc.vector.tensor_scalar(
                            row[:, g * GROUP : (g + 1) * GROUP],
                            pt[:],
                            norm_sb[:, ci : ci + 1],
                            None,
                            op0=mybir.AluOpType.add,
                            op1=mybir.AluOpType.min,
                            accum_out=tmins[:, g : g + 1],
                        )
                    gm = small.tile([128, 1], f32, tag=f"gm{tagp}")
                    nc.vector.tensor_reduce(
                        gm[:], tmins[:], mybir.AxisListType.X, mybir.AluOpType.min
                    )
                    gm8 = small.tile([128, 8], f32, tag=f"gm8{tagp}")
                    nc.any.tensor_copy(out=gm8[:], in_=gm.to_broadcast([128, 8]))
                    i8 = small.tile([128, 8], u32, tag=f"i8{tagp}")
                    nc.vector.max_index(out=i8[:], in_max=gm8[:], in_values=row[:])
                    nc.any.tensor_copy(out=min_sb[:, ci : ci + 1], in_=gm[:])
                    nc.any.tensor_copy(out=idx_sb[:, ci : ci + 1], in_=i8[:, 0:1])

            # per-gt rows: min/argmin over preds (final)
            direction(nsl // 128, ga_repL, pa_repR, m, rx_sb, gmin_sb, gidx_sb, "g")
            # per-pred rows: min/argmin over the gts slice (partial)
            direction(m // 128, pa_repL, ga_repR, nsl, ry_sb, pmin_sb, pidx_sb, "p")

            nc.sync.dma_start(gmin_o[:], gmin_sb[:])
            nc.sync.dma_start(gidx_o[:], gidx_sb[:])
            nc.sync.dma_start(pmin_o[:], pmin_sb[:])
            nc.sync.dma_start(pidx_o[:], pidx_sb[:])

            rep_loop.__exit__(None, None, None)
    nc.finalize()
    return nc


@functools.lru_cache(maxsize=None)
def _get_nc(nsl, m, reps=1):
    return _build_nc(nsl, m, reps)


def _augment(preds_b, gts_bh):
    """Operands for the K=4 scheme.

    ga rows: [-2x0, -2x1, -2x2, 1]  (gts slice, [4, nsl])
    pa rows: [y0, y1, y2, ry]       (full preds, [4, m])
    matmul: Q[n, m] = -2<x_n, y_m> + ry[m];  P = Q + rx via tensor_scalar.
    rxc/ryc: norms laid out [128, len//128] column-per-chunk.
    """
    x = np.ascontiguousarray(gts_bh, dtype=np.float32)
    y = np.ascontiguousarray(preds_b, dtype=np.float32)
    nsl = x.shape[0]
    m = y.shape[0]
    rx = (x[:, 0] * x[:, 0] + x[:, 1] * x[:, 1] + x[:, 2] * x[:, 2]).astype(np.float32)
    ry = (y[:, 0] * y[:, 0] + y[:, 1] * y[:, 1] + y[:, 2] * y[:, 2]).astype(np.float32)
    ga = np.empty((2 * K, nsl), np.float32)
    ga[0:3] = (np.float32(-2.0) * x).T   # lhsT rows, per-gt
    ga[3] = 1.0
    ga[4:7] = ga[0:3]                    # rhs rows, per-pred
    ga[7] = rx
    pa = np.empty((2 * K, m), np.float32)
    pa[0:3] = y.T                        # rhs rows, per-gt
    pa[3] = ry
    pa[4:7] = y.T                        # lhsT rows, per-pred
    pa[7] = 1.0
    rxc = np.ascontiguousarray(rx.reshape(nsl // 128, 128).T)
    ryc = np.ascontiguousarray(ry.reshape(m // 128, 128).T)
    return ga, pa, rxc, ryc


@functools.lru_cache(maxsize=None)
def _get_dispatcher(nsl, m, reps=1):
    """Build the SPMD PJRT dispatcher once and cache it (the stock
    run_bass_via_pjrt re-traces jax.jit on every call)."""
    import jax
    import numpy as _np
    from jax.sharding import Mesh, PartitionSpec
    from jax.experimental.shard_map import shard_map
    import concourse.mybir as mybir
    from concourse import bass2jax

    bass2jax.install_neuronx_cc_hook()
    nc = _get_nc(nsl, m, reps)

    partition_name = nc.partition_id_tensor.name if nc.partition_id_tensor else None
    in_names, out_names, out_avals, zero_outs = [], [], [], []
    for alloc in nc.m.functions[0].allocations:
        if not isinstance(alloc, mybir.MemoryLocationSet):
            continue
        name = alloc.memorylocations[0].name
        if alloc.kind == "ExternalInput":
            if name != partition_name:
                in_names.append(name)
        elif alloc.kind == "ExternalOutput":
            shape = tuple(alloc.tensor_shape)
            dtype = mybir.dt.np(alloc.dtype)
            out_names.append(name)
            out_avals.append(jax.core.ShapedArray(shape, dtype))
            zero_outs.append(_np.zeros(shape, dtype))
    n_params = len(in_names)
    n_outs = len(out_avals)
    all_in_names = list(in_names) + list(out_names)
    if partition_name is not None:
        all_in_names.append(partition_name)
    donate = tuple(range(n_params, n_params + n_outs))

    def _body(*args):
        operands = list(args)
        if partition_name is not None:
            operands.append(bass2jax.partition_id_tensor())
        outs = bass2jax._bass_exec_p.bind(
            *operands,
            out_avals=tuple(out_avals),
            in_names=tuple(all_in_names),
            out_names=tuple(out_names),
            lowering_input_output_aliases=(),
            sim_require_finite=True,
            sim_require_nnan=True,
            nc=nc,
        )
        return tuple(outs)

    devices = jax.devices()[:N_CORES]
    mesh = Mesh(np.asarray(devices), ("core",))
    in_specs = (PartitionSpec("core"),) * (n_params + n_outs)
    out_specs = (PartitionSpec("core"),) * n_outs
    sharded = jax.jit(
        shard_map(_body, mesh=mesh, in_specs=in_specs, out_specs=out_specs,
                  check_rep=False),
        donate_argnums=donate,
        keep_unused=True,
    )

    def dispatch(in_maps):
        concat_in = [
            np.concatenate([np.asarray(in_maps[c][nm]) for c in range(N_CORES)], axis=0)
            for nm in in_names
        ]
        concat_zeros = [
            np.zeros((N_CORES * z.shape[0], *z.shape[1:]), z.dtype) for z in zero_outs
        ]
        out_arrs = sharded(*concat_in, *concat_zeros)
        return [
            {nm: np.asarray(out_arrs[i]).reshape(N_CORES, *out_avals[i].shape)[c]
             for i, nm in enumerate(out_names)}
            for c in range(N_CORES)
        ]

    return dispatch


def _make_in_maps(preds, gts):
    in_maps = []
    for c in range(N_CORES):
        b, h = c // 2, c % 2
        ga, pa, rxc, ryc = _augment(preds[b], gts[b, h * NSL : (h + 1) * NSL])
        in_maps.append({"ga": ga, "pa": pa, "rxc": rxc, "ryc": ryc})
    return in_maps


def kernel(preds, gts, mask):
    preds = np.asarray(preds, dtype=np.float32)
    gts = np.asarray(gts, dtype=np.float32)

    results = _get_dispatcher(NSL, M)(_make_in_maps(preds, gts))

    out_pmin = np.empty((BS, M), np.float32)
    out_gmin = np.empty((BS, N), np.float32)
    out_pidx = np.empty((BS, M), np.int32)
    out_gidx = np.empty((BS, N), np.int32)

    for b in range(BS):
        r0, r1 = results[2 * b], results[2 * b + 1]
        # per-gt rows (min over preds): each half is final
        for h, r in ((0, r0), (1, r1)):
            out_gmin[b, h * NSL : (h + 1) * NSL] = r["gmin"].T.reshape(NSL)
            out_gidx[b, h * NSL : (h + 1) * NSL] = r["gidx"].T.reshape(NSL)
        # per-pred rows: combine the two n-halves
        pm0 = r0["pmin"].T.reshape(M)
        pm1 = r1["pmin"].T.reshape(M)
        pi0 = r0["pidx"].T.reshape(M)
        pi1 = r1["pidx"].T.reshape(M)
        take1 = pm1 < pm0  # tie -> half 0 (lower gt index), first occurrence
        out_pmin[b] = np.where(take1, pm1, pm0)
        out_pidx[b] = np.where(take1, pi1 + NSL, pi0)

    return out_pmin, out_gmin, out_pidx, out_gidx
